# revision 19
# baseline (speedup 1.0000x reference)
"""Trainium2 Bass kernel for capsule attention-routing.

Reference computation (per pixel; 4096 independent problems of shape
[I=32 in-caps, N=32 out-caps, J=16 caps-dim]):
    v[n,j]   = sum_i u[i,n,j]
    cp[i,n]  = sum_j u[i,n,j] * v[n,j] / 4
    c[i,n]   = softmax_n(cp)[i,n] + b[i,n]
    s[n,j]   = sum_i u[i,n,j] * c[i,n]
    out[n,j] = (1 - 1/(exp(|s|_j)+eps)) * s[n,j] / (|s|_j + eps)

Sharding: data-parallel over (batch, h-half): 8 cores x 512 pixels.
Per-core: 8 blocks of 64 pixels, SBUF partitions = (j*8+il), il=i%8.

All tensors stream in fp16; reductions/broadcasts run on the TensorEngine
(0/1/0.25 delta weights, fp16-exact). The four PE passes per block (v-sum,
cp-reduce, c-broadcast, s-sum) each cost stream-of-u (8192 cols); the kernel
is PE-bound, so every other engine is kept under the PE time:
  - u loads halved via fp16 (one 16KB/partition DMA per block)
  - DVE elementwise ops run in 2x_1p mode (all-fp16 operands)
  - the m=u*c multiply splits across Act-copy+DVE-2x / DVE-from-PSUM /
    GpSimd-from-PSUM so no single engine exceeds the PE pass time
  - software pipeline: per iteration i PE runs cp(i), v(i+1), cbc(i)
    interleaved with s(i-1), n2(i-2); keeps PE continuously busy (max
    p-state) with the cb PSUM ring (3 banks) drained in-flight.
Softmax runs without max-subtraction (|cp| <~ 45 safe in fp32 exp; e kept
f32 in SBUF since exp overflows fp16).
"""

import math
import numpy as np
from contextlib import ExitStack

import concourse.bass as bass
import concourse.bacc as bacc
import concourse.tile as tile
import concourse.mybir as mybir
from concourse.bass_utils import run_bass_kernel_spmd

dt = mybir.dt
AF = mybir.ActivationFunctionType
OP = mybir.AluOpType

B, I, N, J, H, W = 4, 32, 32, 16, 32, 32
HW = H * W
NCORES = 8
PIX = B * HW // NCORES      # 512 pixels per core
BLK = 64                    # pixels per block
P16, P8 = 16, 8
SCALE = 1.0 / math.sqrt(16.0)     # 0.25

f32, bf16, f32r = dt.float32, dt.bfloat16, dt.float32r
f16 = dt.float16

# m-stage split per block: of 16 (ib,q) units, 6 via Act-copy + DVE-2x,
# 3 via DVE-direct-from-PSUM, 7 via Pool-direct-from-PSUM.
_POOL_UNITS = {1, 3, 6, 8, 11, 13, 15}
_DVE_UNITS = {2, 7, 12}


def _unit_route(u_ix):
    if u_ix in _POOL_UNITS:
        return "pool"
    if u_ix in _DVE_UNITS:
        return "dve"
    return "act"


K1 = sum(_unit_route(u) == "act" for u in range(16))


def _build_weight_arrays():
    il_of = np.arange(128) % 8          # partition -> il
    j_of = np.arange(128) // 8          # partition -> j

    # v-pass: out[(j2,il2)] = sum_il u[(j,il)] for j==j2 (broadcast over il2)
    wv = np.zeros((128, 128), np.float32)
    for p_in in range(128):
        for p_out in range(128):
            if j_of[p_in] == j_of[p_out]:
                wv[p_in, p_out] = 1.0

    # c-reduce: 16 blocks k=q*4+ib: out[q*32+ib*8+il] = SCALE*sum_j w[(j,il)]
    wc = np.zeros((128, 16 * 128), np.float32)
    for q in range(4):
        for ib in range(4):
            k = q * 4 + ib
            for p_in in range(128):
                wc[p_in, k * 128 + q * 32 + ib * 8 + il_of[p_in]] = SCALE

    # c-bcast: row strips q*32..q*32+32 each hold the same [32,128] pattern.
    # in strip: row (ib2*8+il2), col-block ib: col (j*8+il): delta(ib2==ib, il2==il)
    wcb = np.zeros((128, 4 * 128), np.float32)
    for q in range(4):
        for ib in range(4):
            for il in range(8):
                for j in range(16):
                    wcb[q * 32 + ib * 8 + il, ib * 128 + j * 8 + il] = 1.0

    # s-reduce: 8 blocks q8: out[q8*16+j2] = sum_il m[(j,il)] with j==j2
    ws = np.zeros((128, 8 * 128), np.float32)
    for q8 in range(8):
        for p_in in range(128):
            ws[p_in, q8 * 128 + q8 * 16 + j_of[p_in]] = 1.0

    # norm2: out[(q8b*16+r)] = sum_j ssq[(q8*16+j)] for q8==q8b
    wn = np.zeros((128, 128), np.float32)
    for p_in in range(128):
        for p_out in range(128):
            if p_in // 16 == p_out // 16:
                wn[p_in, p_out] = 1.0

    return {"wv": wv, "wc": wc, "wcb": wcb, "ws": ws, "wn": wn}


def _b_tile_array(b_np):
    # b_t[q*32+ib*8+il, n*16+p] = b[0, ib*8+il, n, 0,0,0]
    bt = np.zeros((128, N * P16), np.float32)
    bsl = b_np.reshape(I, N)
    for q in range(4):
        for ib in range(4):
            for il in range(8):
                row = q * 32 + ib * 8 + il
                bt[row, :] = np.repeat(bsl[ib * 8 + il, :], P16)
    return bt


class _Block:
    """Per-block live tiles (filled in as stages emit)."""

    def __init__(self):
        self.u = None        # [128, (ib,n,p64)] f16 SBUF
        self.v_sb = None     # [128, (n,p64)] f16 SBUF
        self.w = None        # [128, (ib,n,p64)] f16 SBUF
        self.c_ps = None     # [128, (n,p16)] f32 PSUM
        self.e_sb = None     # [128, (n,p16)] f32 SBUF
        self.c_sb = None     # [128, (n,p16)] f16 SBUF
        self.m = [None] * 16  # per (ib,q) unit: [128, (n,p16)] f16 SBUF
        self.spk = None      # [128, (n,p8)] f32 PSUM
        self.ssq = None
        self.n2 = None
        self.norm = None
        self.en = None
        self.rn = None


def _emit(ctx: ExitStack, tc: tile.TileContext, aps: dict, pix: int, with_b: bool):
    nc = tc.nc
    nblk = pix // BLK
    u_d, o_d = aps["u"], aps["out"]

    # constants
    pconst = ctx.enter_context(tc.tile_pool(name="const", bufs=1))
    wv_t = pconst.tile([128, 128], f16, tag="wv")
    wc_t = pconst.tile([128, 16 * 128], f16, tag="wc")
    wcb_t = pconst.tile([128, 4 * 128], f16, tag="wcb")
    ws_t = pconst.tile([128, 8 * 128], f16, tag="ws")
    wn_t = pconst.tile([128, 128], f32r, tag="wn")
    if with_b:
        bt_t = pconst.tile([128, N * P16], f32, tag="bt")

    # pools
    pu = ctx.enter_context(tc.tile_pool(name="u", bufs=5))
    pw = ctx.enter_context(tc.tile_pool(name="w", bufs=2))
    pvsb = ctx.enter_context(tc.tile_pool(name="vsb", bufs=2))
    pesb = ctx.enter_context(tc.tile_pool(name="esb", bufs=2))
    psmall = ctx.enter_context(tc.tile_pool(name="small", bufs=2))
    pcbsb = ctx.enter_context(tc.tile_pool(name="cbsb", bufs=K1 + 2))
    pm = ctx.enter_context(tc.tile_pool(name="m", bufs=20))
    psq = ctx.enter_context(tc.tile_pool(name="sq", bufs=6))

    pvps = ctx.enter_context(tc.tile_pool(name="vps", bufs=2, space="PSUM"))
    pcps = ctx.enter_context(tc.tile_pool(name="cps", bufs=1, space="PSUM"))
    pcb = ctx.enter_context(tc.tile_pool(name="cb", bufs=3, space="PSUM"))
    pspk = ctx.enter_context(tc.tile_pool(name="spk", bufs=1, space="PSUM"))
    pn2 = ctx.enter_context(tc.tile_pool(name="n2", bufs=1, space="PSUM"))

    blocks = [_Block() for _ in range(nblk)]

    # ---- stage emitters ----

    def dma_u(i, chunked=False, first_n=None, rest=False):
        if rest:
            t = blocks[i].u
        else:
            t = pu.tile([128, 4 * N * BLK], f16, tag="T", name="T")
            blocks[i].u = t
        if chunked:
            # v-stage-granular chunks so early blocks' v-pass starts sooner
            u4 = u_d[i].rearrange("P (ib n p) -> P ib n p", ib=4, p=BLK)
            t4 = t[:].rearrange("P (ib n p) -> P ib n p", ib=4, p=BLK)
            sts = range(0, first_n or 4) if not rest else range(1, 4)
            for st in sts:
                nc.sync.dma_start(
                    t4[:, :, st * 8 : (st + 1) * 8, :],
                    u4[:, :, st * 8 : (st + 1) * 8, :],
                )
        else:
            nc.sync.dma_start(t[:], u_d[i])

    def pe_v(i):
        """v-pass + Act copies to v_sb (f16)."""
        bl = blocks[i]
        u3 = bl.u[:].rearrange("P (ib n p) -> P ib n p", ib=4, p=BLK)
        v_sb = pvsb.tile([128, N * BLK], f16, tag="vsb")
        for st in range(4):
            v_ps = pvps.tile([128, 512], f32, tag="vps")
            for ib in range(4):
                nc.tensor.matmul(
                    v_ps[:],
                    wv_t[:],
                    u3[:, ib, st * 8 : (st + 1) * 8, :],
                    start=(ib == 0),
                    stop=(ib == 3),
                )
            nc.scalar.copy(v_sb[:, st * 512 : (st + 1) * 512], v_ps[:])
        bl.v_sb = v_sb

    def pe_cp(i):
        """cp-reduce: 16 matmuls (ib-outer for early-w consumption)."""
        bl = blocks[i]
        c_ps = pcps.tile([128, N * P16], f32, tag="cps")
        c_ps_v = c_ps[:].rearrange("P (n p) -> P n p", p=P16)
        w4 = bl.w[:].rearrange("P (ib n p) -> P ib n p", ib=4, p=BLK)
        # split by n-half so each matmul only needs one w-chunk per ib
        for half in range(2):
            n_sl = slice(half * 16, (half + 1) * 16)
            for ib in range(4):
                for q in range(4):
                    nc.tensor.matmul(
                        c_ps_v[:, n_sl, :],
                        wc_t[:, (q * 4 + ib) * 128 : (q * 4 + ib + 1) * 128],
                        w4[:, ib, n_sl, q * P16 : (q + 1) * P16],
                        start=(ib == 0),
                        stop=(ib == 3),
                        skip_group_check=True,
                    )
        bl.c_ps = c_ps

    def act_exp(i):
        bl = blocks[i]
        e_sb = pesb.tile([128, N * P16], f32, tag="esb")
        nc.scalar.activation(e_sb[:], bl.c_ps[:], AF.Exp)
        bl.e_sb = e_sb

    def soft(i):
        """softmax normalize: z-sum (Pool), 1/z (DVE), e*rz (Pool)."""
        bl = blocks[i]
        z = psmall.tile([128, P16], f32, tag="z")
        nc.vector.tensor_reduce(
            z[:],
            bl.e_sb[:].rearrange("P (n p) -> P p n", p=P16),
            axis=mybir.AxisListType.X,
            op=OP.add,
        )
        rz = psmall.tile([128, P16], f32, tag="rz")
        nc.vector.reciprocal(rz[:], z[:])
        c_sb = psmall.tile([128, N * P16], f16, tag="csb")
        rz_b = rz[:].rearrange("P (o p) -> P o p", o=1).broadcast_to([128, N, P16])
        if with_b:
            c_f = psmall.tile([128, N * P16], f32, tag="cf")
            nc.gpsimd.tensor_tensor(
                c_f[:].rearrange("P (n p) -> P n p", p=P16),
                bl.e_sb[:].rearrange("P (n p) -> P n p", p=P16),
                rz_b,
                op=OP.mult,
            )
            nc.gpsimd.tensor_tensor(c_sb[:], c_f[:], bt_t[:], op=OP.add)
        else:
            nc.gpsimd.tensor_tensor(
                c_sb[:].rearrange("P (n p) -> P n p", p=P16),
                bl.e_sb[:].rearrange("P (n p) -> P n p", p=P16),
                rz_b,
                op=OP.mult,
            )
        bl.c_sb = c_sb

    def pe_cbc_unit(i, u_ix):
        """c-bcast matmul for unit (ib,q) -> cb PSUM tile."""
        bl = blocks[i]
        ib, q = divmod(u_ix, 4)
        cb = pcb.tile([128, N * P16], f32, tag="cb")
        nc.tensor.matmul(
            cb[:].rearrange("P (n p) -> P n p", p=P16),
            wcb_t[q * 32 : (q + 1) * 32, ib * 128 : (ib + 1) * 128],
            bl.c_sb[q * 32 : (q + 1) * 32, :].rearrange("P (n p) -> P n p", p=P16),
            start=True,
            stop=True,
            skip_group_check=True,
            tile_position=(q * 32, 0),
        )
        return cb

    def m_unit(i, u_ix, cb):
        """m[(ib,q)] = u-slice * cb, routed to Act+DVE / DVE / Pool."""
        bl = blocks[i]
        ib, q = divmod(u_ix, 4)
        u_sl = (
            bl.u[:]
            .rearrange("P (ib n p) -> P ib n p", ib=4, p=BLK)[
                :, ib, :, q * P16 : (q + 1) * P16
            ]
        )
        m = pm.tile([128, N * P16], f16, tag="m", name="m_u")
        route = _unit_route(u_ix)
        if route == "act":
            cb_sb = pcbsb.tile([128, N * P16], f16, tag="cbsb")
            nc.scalar.copy(cb_sb[:], cb[:])
            nc.vector.tensor_tensor(
                m[:].rearrange("P (n p) -> P n p", p=P16),
                u_sl,
                cb_sb[:].rearrange("P (n p) -> P n p", p=P16),
                op=OP.mult,
            )
        elif route == "dve":
            nc.vector.tensor_tensor(
                m[:].rearrange("P (n p) -> P n p", p=P16),
                u_sl,
                cb[:].rearrange("P (n p) -> P n p", p=P16),
                op=OP.mult,
            )
        else:
            nc.gpsimd.tensor_tensor(
                m[:].rearrange("P (n p) -> P n p", p=P16),
                u_sl,
                cb[:].rearrange("P (n p) -> P n p", p=P16),
                op=OP.mult,
            )
        bl.m[u_ix] = m

    def pe_s_unit(i, u_ix):
        """two s-reduce matmuls consuming m[u_ix] of block i."""
        bl = blocks[i]
        ib, q = divmod(u_ix, 4)
        if bl.spk is None:
            bl.spk = pspk.tile([128, N * P8], f32, tag="spk", name="spk_t")
        spk_v = bl.spk[:].rearrange("P (n p) -> P n p", p=P8)
        m_v = bl.m[u_ix][:].rearrange("P (n p) -> P n p", p=P16)
        for k2 in range(2):
            q8 = 2 * q + k2
            nc.tensor.matmul(
                spk_v,
                ws_t[:, q8 * 128 : (q8 + 1) * 128],
                m_v[:, :, k2 * P8 : (k2 + 1) * P8],
                start=(u_ix == 0 and k2 == 0),
                stop=(u_ix == 15 and k2 == 1),
                skip_group_check=True,
            )

    def act_square(i):
        bl = blocks[i]
        ssq = psq.tile([128, N * P8], f32r, tag="ssq")
        nc.scalar.activation(ssq[:], bl.spk[:], AF.Square)
        # park s in SBUF f16 so spk's PSUM bank frees this iteration
        s_sb = psq.tile([128, N * P8], f16, tag="s_sb")
        nc.scalar.copy(s_sb[:], bl.spk[:])
        bl.ssq, bl.s_sb = ssq, s_sb

    def pe_n2(i):
        bl = blocks[i]
        n2 = pn2.tile([128, N * P8], f32, tag="n2t", name="n2t")
        nc.tensor.matmul(n2[:], wn_t[:], bl.ssq[:], start=True, stop=True)
        bl.n2 = n2

    def act_sqrt(i):
        """sqrt-table op; lands at iteration end (dep: n2 matmul), so the
        table reload before the next exp-set op falls in Act slack."""
        bl = blocks[i]
        norm = psq.tile([128, N * P8], f32, tag="norm")
        nc.scalar.activation(norm[:], bl.n2[:], AF.Sqrt, bias=1e-30)
        bl.norm = norm

    def act_en(i):
        bl = blocks[i]
        en = psq.tile([128, N * P8], f32, tag="en")
        nc.scalar.activation(en[:], bl.norm[:], AF.Exp, scale=-1.0)
        bl.en = en

    def dve_rn(i):
        bl = blocks[i]
        rn = psq.tile([128, N * P8], f32, tag="rn")
        nc.vector.reciprocal(rn[:], bl.norm[:])
        bl.rn = rn

    def pool_g(i):
        bl = blocks[i]
        g = psq.tile([128, N * P8], f32, tag="g")
        # g = (en - 1) * rn = -(1-en)/norm
        nc.gpsimd.scalar_tensor_tensor(
            g[:], bl.en[:], 1.0, bl.rn[:], op0=OP.subtract, op1=OP.mult
        )
        bl.g = g

    def dve_out(i):
        bl = blocks[i]
        outt = psq.tile([128, N * P8], f16, tag="outt")
        # (-s) * g = s * (1-en)/norm
        nc.vector.scalar_tensor_tensor(
            outt[:], bl.s_sb[:], -1.0, bl.g[:], op0=OP.mult, op1=OP.mult
        )
        nc.scalar.dma_start(o_d[i], outt[:])

    def dve_w_chunk(i, ck):
        """w(i) half-ib chunk: (half, ib) with half-0 chunks first."""
        half, ib = divmod(ck, 4)
        bl = blocks[i]
        if bl.w is None:
            bl.w = pw.tile([128, 4 * N * BLK], f16, tag="w", name="w_t")
        sl = slice(ib * 2048 + half * 1024, ib * 2048 + (half + 1) * 1024)
        nc.vector.tensor_tensor(
            bl.w[:, sl],
            bl.u[:, sl],
            bl.v_sb[:, half * 1024 : (half + 1) * 1024],
            op=OP.mult,
        )

    # ---- pipelined emission ----
    # Iteration i: PE [cp(i), v(i+1), cbc(i)⊗s(i-1), n2(i-1)];
    # DVE [squash-tail(i-2), rz(i), w(i+1)⊗m-units(i)]; Act/Pool follow.
    # startup order: first u(0) chunk, then the v-pass weights, then the
    # rest (each DMA carries ~0.6us of serialized HWDGE overhead).
    dma_u(0, chunked=True, first_n=1)
    nc.sync.dma_start(wv_t[:], aps["wv"])
    dma_u(0, chunked=True, rest=True)
    dma_u(1, chunked=True)
    nc.sync.dma_start(wc_t[:], aps["wc"])
    nc.sync.dma_start(wcb_t[:], aps["wcb"])
    nc.sync.dma_start(ws_t[:], aps["ws"])
    nc.sync.dma_start(wn_t[:], aps["wn"])
    if with_b:
        nc.sync.dma_start(bt_t[:], aps["bt"])
    for i in range(2, nblk):
        dma_u(i)

    pe_v(0)  # prologue
    for ck in range(8):
        dve_w_chunk(0, ck)

    for i in range(nblk + 2):
        has_cur = 0 <= i < nblk


        # squash tail for block i-2 (deps ready since end of iter i-1)
        if 0 <= i - 2 < nblk:
            act_en(i - 2)
            dve_rn(i - 2)
            pool_g(i - 2)
            dve_out(i - 2)

        if has_cur:
            pe_cp(i)
            act_exp(i)
            soft(i)

        if i + 1 < nblk:
            pe_v(i + 1)

        # main interleaved phase: cbc(i) units + s(i-1) matmuls + m(i) + w(i+1)
        w_chunks = list(range(8)) if i + 1 < nblk else []
        for u_ix in range(16):
            if has_cur:
                if u_ix % 2 == 0 and w_chunks:
                    dve_w_chunk(i + 1, w_chunks.pop(0))
                cb = pe_cbc_unit(i, u_ix)
                m_unit(i, u_ix, cb)
            if 0 <= i - 1 < nblk:
                pe_s_unit(i - 1, u_ix)
        for ck in w_chunks:
            dve_w_chunk(i + 1, ck)

        if 0 <= i - 1 < nblk:
            act_square(i - 1)
            pe_n2(i - 1)
            act_sqrt(i - 1)


def encode_u(shard):
    """[I, N, J, pix] -> [nblk][(j,il) part, (ib, n, p64)] f16 device layout."""
    pix = shard.shape[-1]
    nblk = pix // BLK
    a = shard.reshape(4, 8, N, J, nblk, BLK)          # ib, il, n, j, blk, p
    # -> blk, j, il, ib, n, p
    return np.ascontiguousarray(a.transpose(4, 3, 1, 0, 2, 5)).astype(np.float16)


def decode_out(arr, pix):
    """[nblk, 128=(q8,j), N*P8] f16 device layout -> [N, J, pix] f32."""
    nblk = pix // BLK
    a = arr.astype(np.float32).reshape(nblk, 8, J, N, P8)
    return np.ascontiguousarray(a.transpose(3, 2, 0, 1, 4)).reshape(N, J, pix)


_CACHE = {}


def _patch_act_tables():
    """Keep only the act-table sets this kernel uses so a single table load is
    emitted instead of per-block set flip-flops."""
    if getattr(bacc, "_ant_act_tables_patched", False):
        return
    real = bacc.get_activation_tables

    def patched(module_arch):
        tabs = real(module_arch)
        keep = {"natural_log_exp_and_others", "sqrt_and_others"}
        return {
            name: (fns if name in keep else set())
            for name, fns in tabs.items()
        }

    bacc.get_activation_tables = patched
    bacc._ant_act_tables_patched = True


def _get_program(pix, with_b=False):
    key = (pix, with_b)
    if key in _CACHE:
        return _CACHE[key]
    _patch_act_tables()
    nc = bacc.Bacc("TRN2", target_bir_lowering=False, debug=False)
    # register the sqrt-bias constant (per-partition scalar AP)
    _eps_t = nc.alloc_sbuf_tensor("const-f32-eps30", [128, 1], f32)
    nc.gpsimd.memset(_eps_t.ap(), 1e-30)
    nc.const_aps.aps[(f32, 1e-30)] = _eps_t.ap()
    aps = {}
    nblk = pix // BLK
    aps["u"] = nc.dram_tensor(
        "u", [nblk, 128, 4 * N * BLK], f16, kind="ExternalInput"
    ).ap()
    wts = _build_weight_arrays()
    aps["wv"] = nc.dram_tensor("wv", [128, 128], f16, kind="ExternalInput").ap()
    aps["wc"] = nc.dram_tensor("wc", [128, 16 * 128], f16, kind="ExternalInput").ap()
    aps["wcb"] = nc.dram_tensor("wcb", [128, 4 * 128], f16, kind="ExternalInput").ap()
    aps["ws"] = nc.dram_tensor("ws", [128, 8 * 128], f16, kind="ExternalInput").ap()
    aps["wn"] = nc.dram_tensor("wn", [128, 128], f32r, kind="ExternalInput").ap()
    aps["bt"] = nc.dram_tensor("bt", [128, N * P16], f32, kind="ExternalInput").ap()
    aps["out"] = nc.dram_tensor(
        "out", [nblk, 128, N * P8], f16, kind="ExternalOutput"
    ).ap()

    with tile.TileContext(nc) as tc:
        with ExitStack() as ctx:
            _emit(ctx, tc, aps, pix, with_b)
    nc.compile()

    _CACHE[key] = (nc, wts)
    return _CACHE[key]


def kernel(u: np.ndarray, b: np.ndarray) -> np.ndarray:
    u = np.asarray(u, dtype=np.float32)
    b = np.asarray(b, dtype=np.float32)
    with_b = bool(np.any(b))
    nc, wts = _get_program(PIX, with_b=with_b)

    base = {
        "wv": wts["wv"].astype(np.float16),
        "wc": wts["wc"].astype(np.float16),
        "wcb": wts["wcb"].astype(np.float16),
        "ws": wts["ws"].astype(np.float16),
        "wn": wts["wn"],
        "bt": _b_tile_array(b),
    }
    in_maps = []
    for c in range(NCORES):
        bb = c // 2
        h0 = 16 * (c % 2)
        shard = u[bb, :, :, :, h0 : h0 + 16, :].reshape(I, N, J, PIX)
        m = dict(base)
        m["u"] = encode_u(shard)
        in_maps.append(m)

    res = run_bass_kernel_spmd(nc, in_maps, core_ids=list(range(NCORES)))
    out = np.zeros((B, N, J, H, W), np.float32)
    for c in range(NCORES):
        bb = c // 2
        h0 = 16 * (c % 2)
        out[bb, :, :, h0 : h0 + 16, :] = decode_out(
            res.results[c]["out"], PIX
        ).reshape(N, J, 16, W)
    return out


# revision 20
# speedup vs baseline: 1.0013x; 1.0013x over previous
"""Trainium2 Bass kernel for capsule attention-routing.

Reference computation (per pixel; 4096 independent problems of shape
[I=32 in-caps, N=32 out-caps, J=16 caps-dim]):
    v[n,j]   = sum_i u[i,n,j]
    cp[i,n]  = sum_j u[i,n,j] * v[n,j] / 4
    c[i,n]   = softmax_n(cp)[i,n] + b[i,n]
    s[n,j]   = sum_i u[i,n,j] * c[i,n]
    out[n,j] = (1 - 1/(exp(|s|_j)+eps)) * s[n,j] / (|s|_j + eps)

Sharding: data-parallel over (batch, h-half): 8 cores x 512 pixels.
Per-core: 8 blocks of 64 pixels, SBUF partitions = (j*8+il), il=i%8.

All tensors stream in fp16; reductions/broadcasts run on the TensorEngine
(0/1/0.25 delta weights, fp16-exact). The four PE passes per block (v-sum,
cp-reduce, c-broadcast, s-sum) each cost stream-of-u (8192 cols); the kernel
is PE-bound, so every other engine is kept under the PE time:
  - u loads halved via fp16 (one 16KB/partition DMA per block)
  - DVE elementwise ops run in 2x_1p mode (all-fp16 operands)
  - the m=u*c multiply splits across Act-copy+DVE-2x / DVE-from-PSUM /
    GpSimd-from-PSUM so no single engine exceeds the PE pass time
  - software pipeline: per iteration i PE runs cp(i), v(i+1), cbc(i)
    interleaved with s(i-1), n2(i-2); keeps PE continuously busy (max
    p-state) with the cb PSUM ring (3 banks) drained in-flight.
Softmax runs without max-subtraction (|cp| <~ 45 safe in fp32 exp; e kept
f32 in SBUF since exp overflows fp16).
"""

import math
import numpy as np
from contextlib import ExitStack

import concourse.bass as bass
import concourse.bacc as bacc
import concourse.tile as tile
import concourse.mybir as mybir
from concourse.bass_utils import run_bass_kernel_spmd

dt = mybir.dt
AF = mybir.ActivationFunctionType
OP = mybir.AluOpType

B, I, N, J, H, W = 4, 32, 32, 16, 32, 32
HW = H * W
NCORES = 8
PIX = B * HW // NCORES      # 512 pixels per core
BLK = 64                    # pixels per block
P16, P8 = 16, 8
SCALE = 1.0 / math.sqrt(16.0)     # 0.25

f32, bf16, f32r = dt.float32, dt.bfloat16, dt.float32r
f16 = dt.float16

# m-stage split per block: of 16 (ib,q) units, 6 via Act-copy + DVE-2x,
# 3 via DVE-direct-from-PSUM, 7 via Pool-direct-from-PSUM.
_POOL_UNITS = {1, 3, 6, 8, 11, 13, 15}
_DVE_UNITS = {2, 7, 12}


def _unit_route(u_ix):
    if u_ix in _POOL_UNITS:
        return "pool"
    if u_ix in _DVE_UNITS:
        return "dve"
    return "act"


K1 = sum(_unit_route(u) == "act" for u in range(16))


def _build_weight_arrays():
    il_of = np.arange(128) % 8          # partition -> il
    j_of = np.arange(128) // 8          # partition -> j

    # v-pass: out[(j2,il2)] = sum_il u[(j,il)] for j==j2 (broadcast over il2)
    wv = np.zeros((128, 128), np.float32)
    for p_in in range(128):
        for p_out in range(128):
            if j_of[p_in] == j_of[p_out]:
                wv[p_in, p_out] = 1.0

    # c-reduce: 16 blocks k=q*4+ib: out[q*32+ib*8+il] = SCALE*sum_j w[(j,il)]
    wc = np.zeros((128, 16 * 128), np.float32)
    for q in range(4):
        for ib in range(4):
            k = q * 4 + ib
            for p_in in range(128):
                wc[p_in, k * 128 + q * 32 + ib * 8 + il_of[p_in]] = SCALE

    # c-bcast: row strips q*32..q*32+32 each hold the same [32,128] pattern.
    # in strip: row (ib2*8+il2), col-block ib: col (j*8+il): delta(ib2==ib, il2==il)
    wcb = np.zeros((128, 4 * 128), np.float32)
    for q in range(4):
        for ib in range(4):
            for il in range(8):
                for j in range(16):
                    wcb[q * 32 + ib * 8 + il, ib * 128 + j * 8 + il] = 1.0

    # s-reduce: 8 blocks q8: out[q8*16+j2] = sum_il m[(j,il)] with j==j2
    ws = np.zeros((128, 8 * 128), np.float32)
    for q8 in range(8):
        for p_in in range(128):
            ws[p_in, q8 * 128 + q8 * 16 + j_of[p_in]] = 1.0

    # norm2: out[(q8b*16+r)] = sum_j ssq[(q8*16+j)] for q8==q8b
    wn = np.zeros((128, 128), np.float32)
    for p_in in range(128):
        for p_out in range(128):
            if p_in // 16 == p_out // 16:
                wn[p_in, p_out] = 1.0

    return {"wv": wv, "wc": wc, "wcb": wcb, "ws": ws, "wn": wn}


def _b_tile_array(b_np):
    # b_t[q*32+ib*8+il, n*16+p] = b[0, ib*8+il, n, 0,0,0]
    bt = np.zeros((128, N * P16), np.float32)
    bsl = b_np.reshape(I, N)
    for q in range(4):
        for ib in range(4):
            for il in range(8):
                row = q * 32 + ib * 8 + il
                bt[row, :] = np.repeat(bsl[ib * 8 + il, :], P16)
    return bt


class _Block:
    """Per-block live tiles (filled in as stages emit)."""

    def __init__(self):
        self.u = None        # [128, (ib,n,p64)] f16 SBUF
        self.v_sb = None     # [128, (n,p64)] f16 SBUF
        self.w = None        # [128, (ib,n,p64)] f16 SBUF
        self.c_ps = None     # [128, (n,p16)] f32 PSUM
        self.e_sb = None     # [128, (n,p16)] f32 SBUF
        self.c_sb = None     # [128, (n,p16)] f16 SBUF
        self.m = [None] * 16  # per (ib,q) unit: [128, (n,p16)] f16 SBUF
        self.spk = None      # [128, (n,p8)] f32 PSUM
        self.ssq = None
        self.n2 = None
        self.norm = None
        self.en = None
        self.rn = None


def _emit(ctx: ExitStack, tc: tile.TileContext, aps: dict, pix: int, with_b: bool):
    nc = tc.nc
    nblk = pix // BLK
    u_d, o_d = aps["u"], aps["out"]

    # constants
    pconst = ctx.enter_context(tc.tile_pool(name="const", bufs=1))
    wv_t = pconst.tile([128, 128], f16, tag="wv")
    wc_t = pconst.tile([128, 16 * 128], f16, tag="wc")
    wcb_t = pconst.tile([128, 4 * 128], f16, tag="wcb")
    ws_t = pconst.tile([128, 8 * 128], f16, tag="ws")
    wn_t = pconst.tile([128, 128], f32r, tag="wn")
    if with_b:
        bt_t = pconst.tile([128, N * P16], f32, tag="bt")

    # pools
    pu = ctx.enter_context(tc.tile_pool(name="u", bufs=5))
    pw = ctx.enter_context(tc.tile_pool(name="w", bufs=2))
    pvsb = ctx.enter_context(tc.tile_pool(name="vsb", bufs=2))
    pesb = ctx.enter_context(tc.tile_pool(name="esb", bufs=2))
    psmall = ctx.enter_context(tc.tile_pool(name="small", bufs=2))
    pcbsb = ctx.enter_context(tc.tile_pool(name="cbsb", bufs=K1 + 2))
    pm = ctx.enter_context(tc.tile_pool(name="m", bufs=20))
    psq = ctx.enter_context(tc.tile_pool(name="sq", bufs=6))

    pvps = ctx.enter_context(tc.tile_pool(name="vps", bufs=2, space="PSUM"))
    pcps = ctx.enter_context(tc.tile_pool(name="cps", bufs=1, space="PSUM"))
    pcb = ctx.enter_context(tc.tile_pool(name="cb", bufs=3, space="PSUM"))
    pspk = ctx.enter_context(tc.tile_pool(name="spk", bufs=1, space="PSUM"))
    pn2 = ctx.enter_context(tc.tile_pool(name="n2", bufs=1, space="PSUM"))

    blocks = [_Block() for _ in range(nblk)]

    # ---- stage emitters ----

    def dma_u(i, chunked=False, first_n=None, rest=False):
        if rest:
            t = blocks[i].u
        else:
            t = pu.tile([128, 4 * N * BLK], f16, tag="T", name="T")
            blocks[i].u = t
        if chunked:
            # v-stage-granular chunks so early blocks' v-pass starts sooner
            u4 = u_d[i].rearrange("P (ib n p) -> P ib n p", ib=4, p=BLK)
            t4 = t[:].rearrange("P (ib n p) -> P ib n p", ib=4, p=BLK)
            sts = range(0, first_n or 4) if not rest else range(1, 4)
            for st in sts:
                nc.sync.dma_start(
                    t4[:, :, st * 8 : (st + 1) * 8, :],
                    u4[:, :, st * 8 : (st + 1) * 8, :],
                )
        else:
            nc.sync.dma_start(t[:], u_d[i])

    def pe_v(i):
        """v-pass + Act copies to v_sb (f16)."""
        bl = blocks[i]
        u3 = bl.u[:].rearrange("P (ib n p) -> P ib n p", ib=4, p=BLK)
        v_sb = pvsb.tile([128, N * BLK], f16, tag="vsb")
        for st in range(4):
            v_ps = pvps.tile([128, 512], f32, tag="vps")
            for ib in range(4):
                nc.tensor.matmul(
                    v_ps[:],
                    wv_t[:],
                    u3[:, ib, st * 8 : (st + 1) * 8, :],
                    start=(ib == 0),
                    stop=(ib == 3),
                )
            nc.scalar.copy(v_sb[:, st * 512 : (st + 1) * 512], v_ps[:])
        bl.v_sb = v_sb

    def pe_cp(i):
        """cp-reduce: 16 matmuls (ib-outer for early-w consumption)."""
        bl = blocks[i]
        c_ps = pcps.tile([128, N * P16], f32, tag="cps")
        c_ps_v = c_ps[:].rearrange("P (n p) -> P n p", p=P16)
        w4 = bl.w[:].rearrange("P (ib n p) -> P ib n p", ib=4, p=BLK)
        # split by n-half so each matmul only needs one w-chunk per ib
        for half in range(2):
            n_sl = slice(half * 16, (half + 1) * 16)
            for ib in range(4):
                for q in range(4):
                    nc.tensor.matmul(
                        c_ps_v[:, n_sl, :],
                        wc_t[:, (q * 4 + ib) * 128 : (q * 4 + ib + 1) * 128],
                        w4[:, ib, n_sl, q * P16 : (q + 1) * P16],
                        start=(ib == 0),
                        stop=(ib == 3),
                        skip_group_check=True,
                    )
        bl.c_ps = c_ps

    def act_exp(i):
        bl = blocks[i]
        e_sb = pesb.tile([128, N * P16], f32, tag="esb")
        nc.scalar.activation(e_sb[:], bl.c_ps[:], AF.Exp)
        bl.e_sb = e_sb

    def soft(i):
        """softmax normalize: z-sum (Pool), 1/z (DVE), e*rz (Pool)."""
        bl = blocks[i]
        z = psmall.tile([128, P16], f32, tag="z")
        nc.vector.tensor_reduce(
            z[:],
            bl.e_sb[:].rearrange("P (n p) -> P p n", p=P16),
            axis=mybir.AxisListType.X,
            op=OP.add,
        )
        rz = psmall.tile([128, P16], f32, tag="rz")
        nc.vector.reciprocal(rz[:], z[:])
        c_sb = psmall.tile([128, N * P16], f16, tag="csb")
        rz_b = rz[:].rearrange("P (o p) -> P o p", o=1).broadcast_to([128, N, P16])
        if with_b:
            c_f = psmall.tile([128, N * P16], f32, tag="cf")
            nc.gpsimd.tensor_tensor(
                c_f[:].rearrange("P (n p) -> P n p", p=P16),
                bl.e_sb[:].rearrange("P (n p) -> P n p", p=P16),
                rz_b,
                op=OP.mult,
            )
            nc.gpsimd.tensor_tensor(c_sb[:], c_f[:], bt_t[:], op=OP.add)
        else:
            nc.gpsimd.tensor_tensor(
                c_sb[:].rearrange("P (n p) -> P n p", p=P16),
                bl.e_sb[:].rearrange("P (n p) -> P n p", p=P16),
                rz_b,
                op=OP.mult,
            )
        bl.c_sb = c_sb

    def pe_cbc_unit(i, u_ix):
        """c-bcast matmul for unit (ib,q) -> cb PSUM tile."""
        bl = blocks[i]
        ib, q = divmod(u_ix, 4)
        cb = pcb.tile([128, N * P16], f32, tag="cb")
        nc.tensor.matmul(
            cb[:].rearrange("P (n p) -> P n p", p=P16),
            wcb_t[q * 32 : (q + 1) * 32, ib * 128 : (ib + 1) * 128],
            bl.c_sb[q * 32 : (q + 1) * 32, :].rearrange("P (n p) -> P n p", p=P16),
            start=True,
            stop=True,
            skip_group_check=True,
            tile_position=(q * 32, 0),
        )
        return cb

    def m_unit(i, u_ix, cb):
        """m[(ib,q)] = u-slice * cb, routed to Act+DVE / DVE / Pool."""
        bl = blocks[i]
        ib, q = divmod(u_ix, 4)
        u_sl = (
            bl.u[:]
            .rearrange("P (ib n p) -> P ib n p", ib=4, p=BLK)[
                :, ib, :, q * P16 : (q + 1) * P16
            ]
        )
        m = pm.tile([128, N * P16], f16, tag="m", name="m_u")
        route = _unit_route(u_ix)
        if route == "act":
            cb_sb = pcbsb.tile([128, N * P16], f16, tag="cbsb")
            nc.scalar.copy(cb_sb[:], cb[:])
            nc.vector.tensor_tensor(
                m[:].rearrange("P (n p) -> P n p", p=P16),
                u_sl,
                cb_sb[:].rearrange("P (n p) -> P n p", p=P16),
                op=OP.mult,
            )
        elif route == "dve":
            nc.vector.tensor_tensor(
                m[:].rearrange("P (n p) -> P n p", p=P16),
                u_sl,
                cb[:].rearrange("P (n p) -> P n p", p=P16),
                op=OP.mult,
            )
        else:
            nc.gpsimd.tensor_tensor(
                m[:].rearrange("P (n p) -> P n p", p=P16),
                u_sl,
                cb[:].rearrange("P (n p) -> P n p", p=P16),
                op=OP.mult,
            )
        bl.m[u_ix] = m

    def pe_s_unit(i, u_ix):
        """two s-reduce matmuls consuming m[u_ix] of block i."""
        bl = blocks[i]
        ib, q = divmod(u_ix, 4)
        if bl.spk is None:
            bl.spk = pspk.tile([128, N * P8], f32, tag="spk", name="spk_t")
        spk_v = bl.spk[:].rearrange("P (n p) -> P n p", p=P8)
        m_v = bl.m[u_ix][:].rearrange("P (n p) -> P n p", p=P16)
        for k2 in range(2):
            q8 = 2 * q + k2
            nc.tensor.matmul(
                spk_v,
                ws_t[:, q8 * 128 : (q8 + 1) * 128],
                m_v[:, :, k2 * P8 : (k2 + 1) * P8],
                start=(u_ix == 0 and k2 == 0),
                stop=(u_ix == 15 and k2 == 1),
                skip_group_check=True,
            )

    def act_square(i):
        bl = blocks[i]
        ssq = psq.tile([128, N * P8], f32r, tag="ssq")
        nc.scalar.activation(ssq[:], bl.spk[:], AF.Square)
        # park s in SBUF f16 so spk's PSUM bank frees this iteration
        s_sb = psq.tile([128, N * P8], f16, tag="s_sb")
        nc.scalar.copy(s_sb[:], bl.spk[:])
        bl.ssq, bl.s_sb = ssq, s_sb

    def pe_n2(i):
        bl = blocks[i]
        n2 = pn2.tile([128, N * P8], f32, tag="n2t", name="n2t")
        nc.tensor.matmul(n2[:], wn_t[:], bl.ssq[:], start=True, stop=True)
        bl.n2 = n2

    def act_sqrt(i):
        """sqrt-table op; lands at iteration end (dep: n2 matmul), so the
        table reload before the next exp-set op falls in Act slack."""
        bl = blocks[i]
        norm = psq.tile([128, N * P8], f32, tag="norm")
        nc.scalar.activation(norm[:], bl.n2[:], AF.Sqrt, bias=1e-30)
        bl.norm = norm

    def act_en(i):
        bl = blocks[i]
        en = psq.tile([128, N * P8], f32, tag="en")
        nc.scalar.activation(en[:], bl.norm[:], AF.Exp, scale=-1.0)
        bl.en = en

    def dve_rn(i):
        bl = blocks[i]
        rn = psq.tile([128, N * P8], f32, tag="rn")
        nc.vector.reciprocal(rn[:], bl.norm[:])
        bl.rn = rn

    def pool_g(i):
        bl = blocks[i]
        g = psq.tile([128, N * P8], f32, tag="g")
        # g = (en - 1) * rn = -(1-en)/norm
        nc.gpsimd.scalar_tensor_tensor(
            g[:], bl.en[:], 1.0, bl.rn[:], op0=OP.subtract, op1=OP.mult
        )
        bl.g = g

    def dve_out(i):
        bl = blocks[i]
        outt = psq.tile([128, N * P8], f16, tag="outt")
        # (-s) * g = s * (1-en)/norm
        nc.vector.scalar_tensor_tensor(
            outt[:], bl.s_sb[:], -1.0, bl.g[:], op0=OP.mult, op1=OP.mult
        )
        nc.scalar.dma_start(o_d[i], outt[:])

    def dve_w_chunk(i, ck):
        """w(i) half-ib chunk: (half, ib) with half-0 chunks first."""
        half, ib = divmod(ck, 4)
        bl = blocks[i]
        if bl.w is None:
            bl.w = pw.tile([128, 4 * N * BLK], f16, tag="w", name="w_t")
        sl = slice(ib * 2048 + half * 1024, ib * 2048 + (half + 1) * 1024)
        nc.vector.tensor_tensor(
            bl.w[:, sl],
            bl.u[:, sl],
            bl.v_sb[:, half * 1024 : (half + 1) * 1024],
            op=OP.mult,
        )

    # ---- pipelined emission ----
    # Iteration i: PE [cp(i), v(i+1), cbc(i)⊗s(i-1), n2(i-1)];
    # DVE [squash-tail(i-2), rz(i), w(i+1)⊗m-units(i)]; Act/Pool follow.
    # startup order: first u(0) chunk, then the v-pass weights, then the
    # rest (each DMA carries ~0.6us of serialized HWDGE overhead).
    dma_u(0, chunked=True, first_n=1)
    nc.sync.dma_start(wv_t[:], aps["wv"])
    dma_u(0, chunked=True, rest=True)
    dma_u(1)
    nc.sync.dma_start(wc_t[:], aps["wc"])
    nc.sync.dma_start(wcb_t[:], aps["wcb"])
    nc.sync.dma_start(ws_t[:], aps["ws"])
    nc.sync.dma_start(wn_t[:], aps["wn"])
    if with_b:
        nc.sync.dma_start(bt_t[:], aps["bt"])
    for i in range(2, nblk):
        dma_u(i)

    pe_v(0)  # prologue
    for ck in range(8):
        dve_w_chunk(0, ck)

    for i in range(nblk + 2):
        has_cur = 0 <= i < nblk


        # squash tail for block i-2 (deps ready since end of iter i-1)
        if 0 <= i - 2 < nblk:
            act_en(i - 2)
            dve_rn(i - 2)
            pool_g(i - 2)
            dve_out(i - 2)

        if has_cur:
            pe_cp(i)
            act_exp(i)
            soft(i)

        if i + 1 < nblk:
            pe_v(i + 1)

        # main interleaved phase: cbc(i) units + s(i-1) matmuls + m(i) + w(i+1)
        w_chunks = list(range(8)) if i + 1 < nblk else []
        for u_ix in range(16):
            if has_cur:
                if u_ix % 2 == 0 and w_chunks:
                    dve_w_chunk(i + 1, w_chunks.pop(0))
                cb = pe_cbc_unit(i, u_ix)
                m_unit(i, u_ix, cb)
            if 0 <= i - 1 < nblk:
                pe_s_unit(i - 1, u_ix)
        for ck in w_chunks:
            dve_w_chunk(i + 1, ck)

        if 0 <= i - 1 < nblk:
            act_square(i - 1)
            pe_n2(i - 1)
            act_sqrt(i - 1)


def encode_u(shard):
    """[I, N, J, pix] -> [nblk][(j,il) part, (ib, n, p64)] f16 device layout."""
    pix = shard.shape[-1]
    nblk = pix // BLK
    a = shard.reshape(4, 8, N, J, nblk, BLK)          # ib, il, n, j, blk, p
    # -> blk, j, il, ib, n, p
    return np.ascontiguousarray(a.transpose(4, 3, 1, 0, 2, 5)).astype(np.float16)


def decode_out(arr, pix):
    """[nblk, 128=(q8,j), N*P8] f16 device layout -> [N, J, pix] f32."""
    nblk = pix // BLK
    a = arr.astype(np.float32).reshape(nblk, 8, J, N, P8)
    return np.ascontiguousarray(a.transpose(3, 2, 0, 1, 4)).reshape(N, J, pix)


_CACHE = {}


def _patch_act_tables():
    """Keep only the act-table sets this kernel uses so a single table load is
    emitted instead of per-block set flip-flops."""
    if getattr(bacc, "_ant_act_tables_patched", False):
        return
    real = bacc.get_activation_tables

    def patched(module_arch):
        tabs = real(module_arch)
        keep = {"natural_log_exp_and_others", "sqrt_and_others"}
        return {
            name: (fns if name in keep else set())
            for name, fns in tabs.items()
        }

    bacc.get_activation_tables = patched
    bacc._ant_act_tables_patched = True


def _get_program(pix, with_b=False):
    key = (pix, with_b)
    if key in _CACHE:
        return _CACHE[key]
    _patch_act_tables()
    nc = bacc.Bacc("TRN2", target_bir_lowering=False, debug=False)
    # register the sqrt-bias constant (per-partition scalar AP)
    _eps_t = nc.alloc_sbuf_tensor("const-f32-eps30", [128, 1], f32)
    nc.gpsimd.memset(_eps_t.ap(), 1e-30)
    nc.const_aps.aps[(f32, 1e-30)] = _eps_t.ap()
    aps = {}
    nblk = pix // BLK
    aps["u"] = nc.dram_tensor(
        "u", [nblk, 128, 4 * N * BLK], f16, kind="ExternalInput"
    ).ap()
    wts = _build_weight_arrays()
    aps["wv"] = nc.dram_tensor("wv", [128, 128], f16, kind="ExternalInput").ap()
    aps["wc"] = nc.dram_tensor("wc", [128, 16 * 128], f16, kind="ExternalInput").ap()
    aps["wcb"] = nc.dram_tensor("wcb", [128, 4 * 128], f16, kind="ExternalInput").ap()
    aps["ws"] = nc.dram_tensor("ws", [128, 8 * 128], f16, kind="ExternalInput").ap()
    aps["wn"] = nc.dram_tensor("wn", [128, 128], f32r, kind="ExternalInput").ap()
    aps["bt"] = nc.dram_tensor("bt", [128, N * P16], f32, kind="ExternalInput").ap()
    aps["out"] = nc.dram_tensor(
        "out", [nblk, 128, N * P8], f16, kind="ExternalOutput"
    ).ap()

    with tile.TileContext(nc) as tc:
        with ExitStack() as ctx:
            _emit(ctx, tc, aps, pix, with_b)
    nc.compile()

    _CACHE[key] = (nc, wts)
    return _CACHE[key]


def kernel(u: np.ndarray, b: np.ndarray) -> np.ndarray:
    u = np.asarray(u, dtype=np.float32)
    b = np.asarray(b, dtype=np.float32)
    with_b = bool(np.any(b))
    nc, wts = _get_program(PIX, with_b=with_b)

    base = {
        "wv": wts["wv"].astype(np.float16),
        "wc": wts["wc"].astype(np.float16),
        "wcb": wts["wcb"].astype(np.float16),
        "ws": wts["ws"].astype(np.float16),
        "wn": wts["wn"],
        "bt": _b_tile_array(b),
    }
    in_maps = []
    for c in range(NCORES):
        bb = c // 2
        h0 = 16 * (c % 2)
        shard = u[bb, :, :, :, h0 : h0 + 16, :].reshape(I, N, J, PIX)
        m = dict(base)
        m["u"] = encode_u(shard)
        in_maps.append(m)

    res = run_bass_kernel_spmd(nc, in_maps, core_ids=list(range(NCORES)))
    out = np.zeros((B, N, J, H, W), np.float32)
    for c in range(NCORES):
        bb = c // 2
        h0 = 16 * (c % 2)
        out[bb, :, :, h0 : h0 + 16, :] = decode_out(
            res.results[c]["out"], PIX
        ).reshape(N, J, 16, W)
    return out
